# revision 11
# baseline (speedup 1.0000x reference)
"""VQ-VAE nearest-embedding (forward) on 8 Trainium2 NeuronCores.

Full inputs: x (64, 256, 32, 32) f32, weight (256, 512) f32.
Returns (quantized (64, 256, 32, 32) f32, argmin (64, 32, 32) i32),
matching the jax reference:
    dist   = ||x||^2 - 2 x.e + ||e||^2   over K=512 codewords
    argmin = argmin_k dist               (first-min tie break)
    quant  = weight[:, argmin]

Sharding: data-parallel over batch, 8 batches per core. The codebook is
replicated (it is tiny). No collectives are needed in forward.

Per-core pipeline, for each of 64 (batch, 128-pixel) tiles:
  PE   : score_psum[n,k] = x_tile.T @ w            (fp32, 2 accum matmuls)
  DVE  : score = score_psum + (-|e|^2/2)           (argmax(c - e^2/2) == argmin dist)
  DVE  : max8 / max_index  -> idx[n] (u32)
  GPSIMD: indirect DMA gather  quantT[n, :] = weightT[idx[n], :]
  PE   : 2x 128x128 transpose  quantT -> quant[d, n] (psum)
  ACT  : psum -> sbuf quant staging
  DMA  : batch-granular loads of x, stores of quant
"""
import numpy as np

import concourse.bass as bass
import concourse.bacc as bacc
import concourse.mybir as mybir
import concourse.tile as tile
from concourse.bass_utils import run_bass_kernel_spmd
from concourse.masks import make_identity

NCORES = 8
B, D, H, W, K = 64, 256, 32, 32, 512
N = H * W            # 1024 pixels per image
BPC = B // NCORES    # 8 batches per core
NT = N // 128        # 8 n-tiles per batch
f32 = mybir.dt.float32
f32r = mybir.dt.float32r
i32 = mybir.dt.int32
u32 = mybir.dt.uint32

_cache = {}


def build_nc():
    nc = bacc.Bacc("TRN2", target_bir_lowering=False, debug=False,
                   num_devices=NCORES)

    x_d = nc.dram_tensor("x", [BPC, D, N], f32, kind="ExternalInput").ap()
    w_d = nc.dram_tensor("w", [D, K], f32, kind="ExternalInput").ap()
    wt_d = nc.dram_tensor("wt", [K, D], f32, kind="ExternalInput").ap()
    nege2_d = nc.dram_tensor("nege2", [128, K], f32, kind="ExternalInput").ap()

    quant_d = nc.dram_tensor("quant", [BPC, D, N], f32, kind="ExternalOutput").ap()
    argmin_d = nc.dram_tensor("argmin", [BPC * NT, 128], i32, kind="ExternalOutput").ap()

    NTILES = BPC * NT
    LAG = 4   # transposes for tile t issue alongside tile t+LAG's matmuls

    with tile.TileContext(nc) as tc:
        with (
            tc.tile_pool(name="const", bufs=1) as cpool,
            tc.tile_pool(name="xin", bufs=3) as xpool,
            tc.tile_pool(name="qout", bufs=3) as qpool,
            tc.tile_pool(name="score", bufs=8) as spool,
            tc.tile_pool(name="small", bufs=LAG + 6) as mpool,
            tc.tile_pool(name="gat", bufs=LAG + 4) as gpool,
            tc.tile_pool(name="am", bufs=1) as apool,
            tc.tile_pool(name="ps_s", bufs=4, space="PSUM") as ps_s,
            tc.tile_pool(name="ps_t", bufs=3, space="PSUM") as ps_t,
            tc.tile_pool(name="ps_a", bufs=1, space="PSUM") as ps_a,
        ):
            w0 = cpool.tile([128, K], f32)
            w1 = cpool.tile([128, K], f32)
            nege2 = cpool.tile([128, K], f32)
            ident = cpool.tile([128, 128], f32)
            nc.sync.dma_start(w0[:], w_d[0:128, :])
            nc.sync.dma_start(w1[:], w_d[128:256, :])
            nc.sync.dma_start(nege2[:], nege2_d[:])
            make_identity(nc, ident[:])

            # fp32r hi/lo splits of the codebook (one-time, exact to ~2^-23)
            w0h = cpool.tile([128, K], f32r)
            w0l = cpool.tile([128, K], f32r)
            w1h = cpool.tile([128, K], f32r)
            w1l = cpool.tile([128, K], f32r)
            nc.vector.tensor_copy(w0h[:], w0[:])
            nc.vector.tensor_sub(w0l[:], w0[:], w0h[:].bitcast(f32))
            nc.vector.tensor_copy(w1h[:], w1[:])
            nc.vector.tensor_sub(w1l[:], w1[:], w1h[:].bitcast(f32))

            # argmin accumulator: column t = indices (as f32) of n-tile t
            argf = apool.tile([128, NTILES], f32)

            xlos, xhis = {}, {}
            qlos, qhis = {}, {}
            qts = {}

            def issue_batch(b):
                xlo = xpool.tile([128, N], f32, tag="xlo", name=f"xlo{b}")
                xhi = xpool.tile([128, N], f32, tag="xhi", name=f"xhi{b}")
                nc.sync.dma_start(xlo[:, 0:512], x_d[b, 0:128, 0:512])
                nc.sync.dma_start(xlo[:, 512:1024], x_d[b, 0:128, 512:1024])
                nc.sync.dma_start(xhi[:, 0:512], x_d[b, 128:256, 0:512])
                nc.sync.dma_start(xhi[:, 512:1024], x_d[b, 128:256, 512:1024])
                xloh = xpool.tile([128, N], f32r, tag="xloh", name=f"xloh{b}")
                xlol = xpool.tile([128, N], f32r, tag="xlol", name=f"xlol{b}")
                xhih = xpool.tile([128, N], f32r, tag="xhih", name=f"xhih{b}")
                xhil = xpool.tile([128, N], f32r, tag="xhil", name=f"xhil{b}")
                nc.scalar.copy(xloh[:], xlo[:])
                nc.scalar.copy(xhih[:], xhi[:])
                nc.gpsimd.tensor_tensor(
                    out=xlol[:], in0=xlo[:], in1=xloh[:].bitcast(f32),
                    op=mybir.AluOpType.subtract)
                nc.gpsimd.tensor_tensor(
                    out=xhil[:], in0=xhi[:], in1=xhih[:].bitcast(f32),
                    op=mybir.AluOpType.subtract)
                xlos[b] = (xloh, xlol)
                xhis[b] = (xhih, xhil)
                qlos[b] = qpool.tile([128, N], f32, tag="qlo", name=f"qlo{b}")
                qhis[b] = qpool.tile([128, N], f32, tag="qhi", name=f"qhi{b}")

            for it in range(NTILES + LAG):
                # ---- front stage: tile `it` ----
                if it < NTILES:
                    b, t = divmod(it, NT)
                    ns = slice(t * 128, (t + 1) * 128)
                    if it == 0:
                        issue_batch(0)
                    if t == 2 and b + 1 < BPC:
                        issue_batch(b + 1)   # prefetch next batch's loads+splits

                    # distances (negated, shifted): score = x.e - |e|^2/2
                    # fp32-exact via fp32r hi/lo 3-term split per d-tile
                    xloh, xlol = xlos[b]
                    xhih, xhil = xhis[b]
                    ps = ps_s.tile([128, K], f32, tag="score_ps")
                    nc.tensor.matmul(ps[:], lhsT=xloh[:, ns], rhs=w0h[:],
                                     start=True, stop=False)
                    nc.tensor.matmul(ps[:], lhsT=xloh[:, ns], rhs=w0l[:],
                                     start=False, stop=False)
                    nc.tensor.matmul(ps[:], lhsT=xlol[:, ns], rhs=w0h[:],
                                     start=False, stop=False)
                    nc.tensor.matmul(ps[:], lhsT=xhih[:, ns], rhs=w1h[:],
                                     start=False, stop=False)
                    nc.tensor.matmul(ps[:], lhsT=xhih[:, ns], rhs=w1l[:],
                                     start=False, stop=False)
                    nc.tensor.matmul(ps[:], lhsT=xhil[:, ns], rhs=w1h[:],
                                     start=False, stop=True)

                    score = spool.tile([128, K], f32, tag="score_sb")
                    nc.vector.tensor_add(score[:], ps[:], nege2[:])

                    mx = mpool.tile([128, 8], f32, tag="mx")
                    mxi = mpool.tile([128, 8], u32, tag="mxi")
                    nc.vector.max(out=mx[:], in_=score[:])
                    nc.vector.max_index(out=mxi[:], in_max=mx[:], in_values=score[:])

                    # argmin indices column (as f32, exact for < 2^24)
                    nc.vector.tensor_copy(argf[:, it:it + 1], mxi[:, 0:1])

                    # gather codewords: quantT[n, :] = wT[idx[n], :]
                    qt = gpool.tile([128, D], f32, tag="qt")
                    nc.gpsimd.indirect_dma_start(
                        out=qt[:],
                        out_offset=None,
                        in_=wt_d[:],
                        in_offset=bass.IndirectOffsetOnAxis(ap=mxi[:, 0:1], axis=0),
                    )
                    qts[it] = qt

                # ---- back stage: tile `it - LAG` (gather has had time) ----
                jt = it - LAG
                if jt >= 0:
                    jb, js = divmod(jt, NT)
                    nsj = slice(js * 128, (js + 1) * 128)
                    qt = qts.pop(jt)
                    pt = ps_t.tile([128, D], f32, tag="tr_ps")
                    nc.tensor.transpose(out=pt[:, 0:128], in_=qt[:, 0:128],
                                        identity=ident[:])
                    nc.tensor.transpose(out=pt[:, 128:256], in_=qt[:, 128:256],
                                        identity=ident[:])
                    nc.scalar.copy(qlos[jb][:, nsj], pt[:, 0:128])
                    nc.scalar.copy(qhis[jb][:, nsj], pt[:, 128:256])
                    if js == NT - 1:
                        nc.sync.dma_start(quant_d[jb, 0:128, :], qlos.pop(jb)[:])
                        nc.sync.dma_start(quant_d[jb, 128:256, :], qhis.pop(jb)[:])

            # argmin: transpose [128, 64] -> [64, 128], cast to i32, store
            pa = ps_a.tile([NTILES, 128], f32)
            nc.tensor.transpose(out=pa[:], in_=argf[:], identity=ident[:])
            ami = apool.tile([NTILES, 128], i32)
            nc.vector.tensor_copy(ami[:], pa[:])
            nc.sync.dma_start(argmin_d[:], ami[:])

    nc.compile()
    return nc


def kernel(x: np.ndarray, weight: np.ndarray):
    x = np.ascontiguousarray(np.asarray(x, dtype=np.float32))
    weight = np.ascontiguousarray(np.asarray(weight, dtype=np.float32))
    assert x.shape == (B, D, H, W) and weight.shape == (D, K)

    if "nc" not in _cache:
        _cache["nc"] = build_nc()
    nc = _cache["nc"]

    e_sq = np.sum(weight * weight, axis=0, dtype=np.float32)   # (K,)
    nege2 = np.tile((-0.5) * e_sq, (128, 1)).astype(np.float32)
    wt = np.ascontiguousarray(weight.T)                        # (K, D)
    xs = x.reshape(B, D, N)

    in_maps = []
    for c in range(NCORES):
        in_maps.append({
            "x": np.ascontiguousarray(xs[c * BPC:(c + 1) * BPC]),
            "w": weight,
            "wt": wt,
            "nege2": nege2,
        })

    res = run_bass_kernel_spmd(nc, in_maps, list(range(NCORES)))

    quant = np.empty((B, D, N), dtype=np.float32)
    argmin = np.empty((B, N), dtype=np.int32)
    for c in range(NCORES):
        r = res.results[c]
        quant[c * BPC:(c + 1) * BPC] = r["quant"]
        argmin[c * BPC:(c + 1) * BPC] = r["argmin"].reshape(BPC, N)

    return quant.reshape(B, D, H, W), argmin.reshape(B, H, W)


# revision 12
# speedup vs baseline: 1.0474x; 1.0474x over previous
"""VQ-VAE nearest-embedding (forward) on 8 Trainium2 NeuronCores.

Full inputs: x (64, 256, 32, 32) f32, weight (256, 512) f32.
Returns (quantized (64, 256, 32, 32) f32, argmin (64, 32, 32) i32),
matching the jax reference:
    dist   = ||x||^2 - 2 x.e + ||e||^2   over K=512 codewords
    argmin = argmin_k dist               (first-min tie break)
    quant  = weight[:, argmin]

Sharding: data-parallel over batch, 8 batches per core. The codebook is
replicated (it is tiny). No collectives are needed in forward.

Per-core pipeline, for each of 64 (batch, 128-pixel) tiles:
  PE   : score_psum[n,k] = x_tile.T @ w            (fp32, 2 accum matmuls)
  DVE  : score = score_psum + (-|e|^2/2)           (argmax(c - e^2/2) == argmin dist)
  DVE  : max8 / max_index  -> idx[n] (u32)
  GPSIMD: indirect DMA gather  quantT[n, :] = weightT[idx[n], :]
  PE   : 2x 128x128 transpose  quantT -> quant[d, n] (psum)
  ACT  : psum -> sbuf quant staging
  DMA  : batch-granular loads of x, stores of quant
"""
import numpy as np

import concourse.bass as bass
import concourse.bacc as bacc
import concourse.mybir as mybir
import concourse.tile as tile
from concourse.bass_utils import run_bass_kernel_spmd
from concourse.masks import make_identity

NCORES = 8
B, D, H, W, K = 64, 256, 32, 32, 512
N = H * W            # 1024 pixels per image
BPC = B // NCORES    # 8 batches per core
NT = N // 128        # 8 n-tiles per batch
f32 = mybir.dt.float32
f32r = mybir.dt.float32r
i32 = mybir.dt.int32
u32 = mybir.dt.uint32

_cache = {}


def build_nc():
    nc = bacc.Bacc("TRN2", target_bir_lowering=False, debug=False,
                   num_devices=NCORES)

    x_d = nc.dram_tensor("x", [BPC, D, N], f32, kind="ExternalInput").ap()
    w_d = nc.dram_tensor("w", [D, K], f32, kind="ExternalInput").ap()
    wt_d = nc.dram_tensor("wt", [K, D], f32, kind="ExternalInput").ap()
    nege2_d = nc.dram_tensor("nege2", [128, K], f32, kind="ExternalInput").ap()

    quant_d = nc.dram_tensor("quant", [BPC, D, N], f32, kind="ExternalOutput").ap()
    argmin_d = nc.dram_tensor("argmin", [BPC * NT, 128], i32, kind="ExternalOutput").ap()

    NTILES = BPC * NT
    LAG = 4   # transposes for tile t issue alongside tile t+LAG's matmuls

    with tile.TileContext(nc) as tc:
        with (
            tc.tile_pool(name="const", bufs=1) as cpool,
            tc.tile_pool(name="xin", bufs=3) as xpool,
            tc.tile_pool(name="qout", bufs=3) as qpool,
            tc.tile_pool(name="score", bufs=8) as spool,
            tc.tile_pool(name="small", bufs=LAG + 6) as mpool,
            tc.tile_pool(name="gat", bufs=LAG + 4) as gpool,
            tc.tile_pool(name="am", bufs=1) as apool,
            tc.tile_pool(name="ps_s", bufs=4, space="PSUM") as ps_s,
            tc.tile_pool(name="ps_t", bufs=3, space="PSUM") as ps_t,
            tc.tile_pool(name="ps_a", bufs=1, space="PSUM") as ps_a,
        ):
            w0 = cpool.tile([128, K], f32)
            w1 = cpool.tile([128, K], f32)
            nege2 = cpool.tile([128, K], f32)
            ident = cpool.tile([128, 128], f32)
            nc.sync.dma_start(w0[:], w_d[0:128, :])
            nc.sync.dma_start(w1[:], w_d[128:256, :])
            nc.sync.dma_start(nege2[:], nege2_d[:])
            make_identity(nc, ident[:])

            # fp32r hi/lo splits of the codebook (one-time, exact to ~2^-23)
            w0h = cpool.tile([128, K], f32r)
            w0l = cpool.tile([128, K], f32r)
            w1h = cpool.tile([128, K], f32r)
            w1l = cpool.tile([128, K], f32r)
            nc.vector.tensor_copy(w0h[:], w0[:])
            nc.vector.tensor_sub(w0l[:], w0[:], w0h[:].bitcast(f32))
            nc.vector.tensor_copy(w1h[:], w1[:])
            nc.vector.tensor_sub(w1l[:], w1[:], w1h[:].bitcast(f32))

            # wide accumulator: max_index writes its 8 outputs per tile here;
            # column ti*8 is the argmin for tile ti
            argfw = apool.tile([128, NTILES * 8], u32)
            argf = apool.tile([128, NTILES], f32)

            xlos, xhis = {}, {}
            qlos, qhis = {}, {}
            qts = {}
            scores = {}

            def issue_batch(b):
                xlo = xpool.tile([128, N], f32, tag="xlo", name=f"xlo{b}")
                xhi = xpool.tile([128, N], f32, tag="xhi", name=f"xhi{b}")
                nc.sync.dma_start(xlo[:, 0:512], x_d[b, 0:128, 0:512])
                nc.sync.dma_start(xlo[:, 512:1024], x_d[b, 0:128, 512:1024])
                nc.sync.dma_start(xhi[:, 0:512], x_d[b, 128:256, 0:512])
                nc.sync.dma_start(xhi[:, 512:1024], x_d[b, 128:256, 512:1024])
                xloh = xpool.tile([128, N], f32r, tag="xloh", name=f"xloh{b}")
                xlol = xpool.tile([128, N], f32r, tag="xlol", name=f"xlol{b}")
                xhih = xpool.tile([128, N], f32r, tag="xhih", name=f"xhih{b}")
                xhil = xpool.tile([128, N], f32r, tag="xhil", name=f"xhil{b}")
                nc.scalar.copy(xloh[:], xlo[:])
                nc.scalar.copy(xhih[:], xhi[:])
                nc.gpsimd.tensor_tensor(
                    out=xlol[:], in0=xlo[:], in1=xloh[:].bitcast(f32),
                    op=mybir.AluOpType.subtract)
                nc.gpsimd.tensor_tensor(
                    out=xhil[:], in0=xhi[:], in1=xhih[:].bitcast(f32),
                    op=mybir.AluOpType.subtract)
                xlos[b] = (xloh, xlol)
                xhis[b] = (xhih, xhil)
                qlos[b] = qpool.tile([128, N], f32, tag="qlo", name=f"qlo{b}")
                qhis[b] = qpool.tile([128, N], f32, tag="qhi", name=f"qhi{b}")

            for it in range(NTILES + LAG + 1):
                # ---- front stage: tile `it` ----
                if it < NTILES:
                    b, t = divmod(it, NT)
                    ns = slice(t * 128, (t + 1) * 128)
                    if it == 0:
                        issue_batch(0)
                    if t == 2 and b + 1 < BPC:
                        issue_batch(b + 1)   # prefetch next batch's loads+splits

                    # distances (negated, shifted): score = x.e - |e|^2/2
                    # fp32-exact via fp32r hi/lo 3-term split per d-tile
                    xloh, xlol = xlos[b]
                    xhih, xhil = xhis[b]
                    ps = ps_s.tile([128, K], f32, tag="score_ps")
                    nc.tensor.matmul(ps[:], lhsT=xloh[:, ns], rhs=w0h[:],
                                     start=True, stop=False)
                    nc.tensor.matmul(ps[:], lhsT=xloh[:, ns], rhs=w0l[:],
                                     start=False, stop=False)
                    nc.tensor.matmul(ps[:], lhsT=xlol[:, ns], rhs=w0h[:],
                                     start=False, stop=False)
                    nc.tensor.matmul(ps[:], lhsT=xhih[:, ns], rhs=w1h[:],
                                     start=False, stop=False)
                    nc.tensor.matmul(ps[:], lhsT=xhih[:, ns], rhs=w1l[:],
                                     start=False, stop=False)
                    nc.tensor.matmul(ps[:], lhsT=xhil[:, ns], rhs=w1h[:],
                                     start=False, stop=True)

                    score = spool.tile([128, K], f32, tag="score_sb")
                    nc.vector.tensor_add(score[:], ps[:], nege2[:])
                    scores[it] = score

                # ---- mid stage: argmin + gather for tile `it - 1` ----
                mt = it - 1
                if 0 <= mt < NTILES:
                    score = scores.pop(mt)
                    mx = mpool.tile([128, 8], f32, tag="mx")
                    nc.vector.max(out=mx[:], in_=score[:])
                    mxi = argfw[:, mt * 8:(mt + 1) * 8]
                    nc.vector.max_index(out=mxi, in_max=mx[:], in_values=score[:])

                    # gather codewords: quantT[n, :] = wT[idx[n], :]
                    qt = gpool.tile([128, D], f32, tag="qt")
                    nc.gpsimd.indirect_dma_start(
                        out=qt[:],
                        out_offset=None,
                        in_=wt_d[:],
                        in_offset=bass.IndirectOffsetOnAxis(
                            ap=argfw[:, mt * 8:mt * 8 + 1], axis=0),
                    )
                    qts[mt] = qt

                # ---- back stage: tile `it - LAG` (gather has had time) ----
                jt = it - LAG - 1
                if jt >= 0:
                    jb, js = divmod(jt, NT)
                    nsj = slice(js * 128, (js + 1) * 128)
                    qt = qts.pop(jt)
                    pt = ps_t.tile([128, D], f32, tag="tr_ps")
                    nc.tensor.transpose(out=pt[:, 0:128], in_=qt[:, 0:128],
                                        identity=ident[:])
                    nc.tensor.transpose(out=pt[:, 128:256], in_=qt[:, 128:256],
                                        identity=ident[:])
                    nc.scalar.copy(qlos[jb][:, nsj], pt[:, 0:128])
                    nc.scalar.copy(qhis[jb][:, nsj], pt[:, 128:256])
                    if js == NT - 1:
                        nc.sync.dma_start(quant_d[jb, 0:128, :], qlos.pop(jb)[:])
                        nc.sync.dma_start(quant_d[jb, 128:256, :], qhis.pop(jb)[:])

            # argmin: cast stride-8 view, transpose [128, 64] -> [64, 128], store
            nc.vector.tensor_copy(
                argf[:], argfw[:].rearrange("p (t e) -> p t e", e=8)[:, :, 0])
            pa = ps_a.tile([NTILES, 128], f32)
            nc.tensor.transpose(out=pa[:], in_=argf[:], identity=ident[:])
            ami = apool.tile([NTILES, 128], i32)
            nc.vector.tensor_copy(ami[:], pa[:])
            nc.sync.dma_start(argmin_d[:], ami[:])

    nc.compile()
    return nc


def kernel(x: np.ndarray, weight: np.ndarray):
    x = np.ascontiguousarray(np.asarray(x, dtype=np.float32))
    weight = np.ascontiguousarray(np.asarray(weight, dtype=np.float32))
    assert x.shape == (B, D, H, W) and weight.shape == (D, K)

    if "nc" not in _cache:
        _cache["nc"] = build_nc()
    nc = _cache["nc"]

    e_sq = np.sum(weight * weight, axis=0, dtype=np.float32)   # (K,)
    nege2 = np.tile((-0.5) * e_sq, (128, 1)).astype(np.float32)
    wt = np.ascontiguousarray(weight.T)                        # (K, D)
    xs = x.reshape(B, D, N)

    in_maps = []
    for c in range(NCORES):
        in_maps.append({
            "x": np.ascontiguousarray(xs[c * BPC:(c + 1) * BPC]),
            "w": weight,
            "wt": wt,
            "nege2": nege2,
        })

    res = run_bass_kernel_spmd(nc, in_maps, list(range(NCORES)))

    quant = np.empty((B, D, N), dtype=np.float32)
    argmin = np.empty((B, N), dtype=np.int32)
    for c in range(NCORES):
        r = res.results[c]
        quant[c * BPC:(c + 1) * BPC] = r["quant"]
        argmin[c * BPC:(c + 1) * BPC] = r["argmin"].reshape(BPC, N)

    return quant.reshape(B, D, H, W), argmin.reshape(B, H, W)
